# revision 22
# baseline (speedup 1.0000x reference)
"""AttentionBlock (GroupNorm32 + 8-head self-attention + proj + residual) on
8 Trainium2 NeuronCores, data-parallel over the batch (B=8 -> 1 element/core).

kernel(**inputs) takes the FULL unsharded inputs (numpy) and returns the FULL
output [8, 512, 32, 32].

Per-core device program (C=512 ch, N=1024 px, 8 heads, hd=64):
  xn  = (x - mean_g) * rsqrt(var_g + eps)        GroupNorm, gamma/beta folded
                                                 into qkv weights on the host
  q,k = Wqk xn + b   (q pre-scaled by 1/8)       [ch-on-partitions layout]
  vT  = xn^T Wv^T    (+ ones column)             [px-on-partitions layout]
  P   = exp(k_h^T q_h)  (|logits| < 7 -> no max subtraction needed)
  o,Z = vT_h^T P        (ones column of vT yields the softmax denominator Z)
  ao  = o * (1/Z)       (1/Z via reciprocal_approx_fast + DMA bcast)
  y   = x + projW ao + proj_b'                   (v-bias folded through proj)

Dtypes: weights/xn/q/k/ao in bf16 (halves the input DMA and LDWEIGHTS time);
logits, P, vT and all PSUM accumulation in fp32/f32r; GroupNorm stats and the
residual path in fp32.

Structure notes:
  - attention-output and Z rows are copied out of PSUM right after each
    head's PV accumulation (high priority), so PSUM recycles and the PE
    never stalls (stalls trip the HAM clock throttle to 1.2 GHz).
  - 1/Z: [1,512] reciprocal_approx_fast per (head, chunk) + a DRAM-bounce
    partition broadcast, multiplied into ao in place.
  - x is striped over all 3 DMA queues first; bf16 weights follow.
  - warm-up matmuls during the GroupNorm prologue spin the PE up to 2.4 GHz
    before the qkv GEMMs start.
"""

import sys

if "/opt/trn_rl_repo" not in sys.path:
    sys.path.insert(0, "/opt/trn_rl_repo")

import numpy as np
import ml_dtypes

import concourse.bass as bass
import concourse.tile as tile
from concourse import bacc, mybir
from concourse.alu_op_type import AluOpType
from concourse.bass_utils import run_bass_kernel_spmd

F32 = mybir.dt.float32
F32R = mybir.dt.float32r
BF16 = mybir.dt.bfloat16
I16 = mybir.dt.int16
AF = mybir.ActivationFunctionType

# Schraudolph exp in bf16 bit-space: int16 bits = round(x*SCH_A + SCH_B),
# reinterpreted as bf16 ~= exp(x) to ~3% (the c=8 offset tuned on the final
# output).  Tiles SCH_JTS of each head's 8 logits tiles run on the DVE this
# way, so the ACT engine stops being the attention pacer.
SCH_A = 128.0 / np.log(2.0)
SCH_B = 127.0 * 128.0 - 8.0
SCH_JTS = (3, 6)
BFNP = ml_dtypes.bfloat16

C = 512          # channels
N = 1024         # pixels (32x32)
NH = 8           # heads
HD = 64          # head dim
GS = 16          # channels per groupnorm group
EPS = 1e-5
CT = C // 128    # channel tiles
JT = N // 128    # pixel tiles
IC = N // 512    # moving chunks
NCORES = 8


def _host_prep(x, gn_gamma, gn_beta, qkv_w, qkv_b, proj_w, proj_b):
    f = np.float32
    gamma = np.asarray(gn_gamma, f)
    beta = np.asarray(gn_beta, f)
    qkv_w = np.asarray(qkv_w, f)
    qkv_b = np.asarray(qkv_b, f)
    proj_w = np.asarray(proj_w, f)
    proj_b = np.asarray(proj_b, f)
    scale = f(HD) ** f(-0.5)

    Wq, Wk, Wv = qkv_w[0:C], qkv_w[C:2 * C], qkv_w[2 * C:3 * C]
    bq = (qkv_b[0:C] + Wq @ beta) * scale
    bk = qkv_b[C:2 * C] + Wk @ beta
    bv = qkv_b[2 * C:3 * C] + Wv @ beta
    Wq = Wq * gamma[None, :] * scale
    Wk = Wk * gamma[None, :]
    Wv = Wv * gamma[None, :]

    A = np.zeros((128, 8), f)
    A[np.arange(128), np.arange(128) // GS] = f(1.0 / GS)
    E = np.zeros((8, 128), f)
    E[np.arange(128) // GS, np.arange(128)] = f(1.0)

    weights = {
        "wqkT": np.ascontiguousarray(np.concatenate([Wq, Wk], 0).T).astype(BFNP),
        "wvT": np.ascontiguousarray(Wv.T).astype(BFNP),
        "qkb": np.concatenate([bq, bk]),
        "pwT": np.ascontiguousarray(proj_w.T).astype(BFNP),
        "pb": proj_b + proj_w @ bv,
        "gA": A, "gE": E,
    }
    xs = [np.ascontiguousarray(np.asarray(x[b], f).reshape(C, N))
          for b in range(x.shape[0])]
    xbs = [xx.astype(BFNP) for xx in xs]
    return weights, xs, xbs


def _declare_io(nc):
    io = {}
    io["x"] = nc.dram_tensor("x", [C, N], F32, kind="ExternalInput")
    io["xb"] = nc.dram_tensor("xb", [C, N], BF16, kind="ExternalInput")
    io["wqkT"] = nc.dram_tensor("wqkT", [C, 2 * C], BF16, kind="ExternalInput")
    io["wvT"] = nc.dram_tensor("wvT", [C, C], BF16, kind="ExternalInput")
    io["qkb"] = nc.dram_tensor("qkb", [2 * C], F32, kind="ExternalInput")
    io["pwT"] = nc.dram_tensor("pwT", [C, C], BF16, kind="ExternalInput")
    io["pb"] = nc.dram_tensor("pb", [C], F32, kind="ExternalInput")
    io["gA"] = nc.dram_tensor("gA", [128, 8], F32, kind="ExternalInput")
    io["gE"] = nc.dram_tensor("gE", [8, 128], F32, kind="ExternalInput")
    io["out"] = nc.dram_tensor("out", [C, N], F32, kind="ExternalOutput")
    return io


def _build(nc, io, p_bufs=17, warmup=8):
    def mm(ap):  # fp32 tensors feeding the PE go as float32r (full rate)
        return ap.bitcast(F32R)

    with tile.TileContext(nc) as tc:
        with (
            tc.tile_pool(name="const", bufs=1) as const,
            tc.tile_pool(name="big", bufs=1) as big,
            tc.tile_pool(name="pp", bufs=p_bufs) as ppool,
            tc.tile_pool(name="sm", bufs=4) as sm,
            tc.tile_pool(name="zs", bufs=6) as zs,
            tc.tile_pool(name="zbp", bufs=4) as zbp,
            tc.tile_pool(name="zdp", bufs=3, space="DRAM") as zdp,
            tc.tile_pool(name="psQ", bufs=2, space=bass.MemorySpace.PSUM) as psQ,
            tc.tile_pool(name="psA", bufs=3, space=bass.MemorySpace.PSUM) as psA,
        ):
            # ---- input loads: x striped over all 3 DMA queues first ------
            qd = [nc.sync, nc.gpsimd, nc.scalar]
            xb_sb, x_sb, wqk_sb, wv_sb, pw_sb = [], [], [], [], []
            # critical path first: bf16 x (GroupNorm+qkv) and wqk; the f32 x
            # (residual only) and the later-phase weights trail behind.
            for t in range(CT):
                xbt = big.tile([128, N], BF16, tag=f"xb{t}", name=f"xb{t}")
                qd[t % 3].dma_start(out=xbt[:],
                                    in_=io["xb"][128 * t:128 * (t + 1), :])
                xb_sb.append(xbt)
            for t in range(CT):
                w1 = const.tile([128, 2 * C], BF16, tag=f"wqk{t}", name=f"wqk{t}")
                qd[(CT + t) % 3].dma_start(
                    out=w1[:], in_=io["wqkT"][128 * t:128 * (t + 1), :])
                wqk_sb.append(w1)
            for t in range(CT):
                w2 = const.tile([128, C], BF16, tag=f"wv{t}", name=f"wv{t}")
                qd[(2 * CT + t) % 3].dma_start(
                    out=w2[:], in_=io["wvT"][128 * t:128 * (t + 1), :])
                wv_sb.append(w2)
            for t in range(CT):
                xt = big.tile([128, N], F32, tag=f"x{t}", name=f"x{t}")
                qd[t % 3].dma_start(out=xt[:], in_=io["x"][128 * t:128 * (t + 1), :])
                x_sb.append(xt)
            for t in range(CT):
                w3 = const.tile([128, C], BF16, tag=f"pw{t}", name=f"pw{t}")
                qd[(3 * CT + t) % 3].dma_start(
                    out=w3[:], in_=io["pwT"][128 * t:128 * (t + 1), :])
                pw_sb.append(w3)

            qkb_sb = const.tile([128, 8], F32, tag="qkb", name="qkb")
            nc.scalar.dma_start(out=qkb_sb[:],
                                in_=io["qkb"][:].rearrange("(t p) -> p t", p=128))
            pb_sb = const.tile([128, 4], F32, tag="pb", name="pb")
            nc.scalar.dma_start(out=pb_sb[:],
                                in_=io["pb"][:].rearrange("(t p) -> p t", p=128))
            A_sb = const.tile([128, 8], F32, tag="gA", name="gA")
            nc.sync.dma_start(out=A_sb[:], in_=io["gA"][:])
            E_sb = const.tile([8, 128], F32, tag="gE", name="gE")
            nc.sync.dma_start(out=E_sb[:], in_=io["gE"][:])
            eps_sb = const.tile([128, 1], F32, tag="eps", name="eps")
            nc.vector.memset(eps_sb[:], EPS)
            ones_sb = const.tile([128, 1], F32, tag="ones", name="ones")
            nc.vector.memset(ones_sb[:], 1.0)

            # ---- PE warm-up during the GroupNorm prologue ---------------
            # junk matmuls on the first weight tile keep the PE busy >3.4us
            # so the HAM clock gate opens (1.2 -> 2.4 GHz) before real GEMMs
            for w in range(warmup):
                pw_ps = psA.tile([128, N], F32, tag="pp", name="warm")
                nc.tensor.matmul(pw_ps[:, 0:512],
                                 lhsT=wqk_sb[0][:, 0:128],
                                 rhs=wqk_sb[0][:, 0:512],
                                 start=True, stop=True)

            # ---- GroupNorm ---------------------------------------------
            # per-channel mean / E[x^2] via bn_stats (free-dim reduction) ...
            stats_all = sm.tile([128, 8], F32, tag="stats_all", name="stats_all")
            for t in range(CT):
                st = sm.tile([128, 2, 6], F32, tag="bnst", name="bnst")
                nc.vector.bn_stats(out=st[:, 0, :], in_=xb_sb[t][:, 0:512])
                nc.vector.bn_stats(out=st[:, 1, :], in_=xb_sb[t][:, 512:1024])
                mv = sm.tile([128, 2], F32, tag="bnmv", name="bnmv")
                nc.vector.bn_aggr(out=mv[:], in_=st[:])
                nc.vector.tensor_copy(out=stats_all[:, 2 * t:2 * t + 1], in_=mv[:, 0:1])
                nc.vector.scalar_tensor_tensor(
                    out=stats_all[:, 2 * t + 1:2 * t + 2],
                    in0=mv[:, 0:1], scalar=mv[:, 0:1], in1=mv[:, 1:2],
                    op0=AluOpType.mult, op1=AluOpType.add)

            # ... then group-aggregate across partitions with a tiny matmul
            ps_g = psQ.tile([8, 8], F32, tag="ps", name="ps")
            nc.tensor.matmul(ps_g[:], lhsT=A_sb[:], rhs=stats_all[:],
                             start=True, stop=True)
            gs = sm.tile([8, 8], F32, tag="gs", name="gs")
            nc.vector.tensor_copy(out=gs[:], in_=ps_g[:])
            gsr = gs[:].rearrange("p (t s) -> p s t", s=2)
            gmean, gex2 = gsr[:, 0, :], gsr[:, 1, :]
            tmp = sm.tile([8, 2, 4], F32, tag="gtmp", name="gtmp")
            nc.vector.tensor_tensor(out=tmp[:, 0, :], in0=gmean, in1=gmean,
                                    op=AluOpType.mult)
            nc.vector.tensor_tensor(out=tmp[:, 1, :], in0=gex2, in1=tmp[:, 0, :],
                                    op=AluOpType.subtract)
            # rstd = rsqrt(var+eps) via the fp32 bit trick + 2 Newton
            # iterations on the DVE ([8,4] values, all tiny ops).  Avoids
            # Ln/Exp here so the only ACT table set ever needed is the
            # preamble-loaded exp set (no mid-kernel ACT_TABLE_LOADs).
            I32 = mybir.dt.int32
            ve = sm.tile([8, 4], F32, tag="lnv", name="ve")
            nc.vector.tensor_scalar_add(ve[:], tmp[:, 1, :], float(EPS))
            fc = sm.tile([8, 4], F32, tag="fc", name="fc")
            nc.vector.tensor_copy(out=fc[:], in_=ve[:].bitcast(I32))
            y0b = sm.tile([8, 4], I32, tag="y0b", name="y0b")
            nc.vector.tensor_scalar(out=y0b[:], in0=fc[:],
                                    scalar1=-0.5, scalar2=1597463007.0,
                                    op0=AluOpType.mult, op1=AluOpType.add)
            hv = sm.tile([8, 4], F32, tag="hv", name="hv")
            nc.vector.tensor_scalar_mul(hv[:], ve[:], 0.5)
            yy = y0b[:].bitcast(F32)
            gm = sm.tile([8, 2, 4], F32, tag="gm", name="gm")
            nt = sm.tile([8, 4, 4], F32, tag="nt", name="nt")
            for it in range(2):
                tsq, ynew = nt[:, 2 * it, :], nt[:, 2 * it + 1, :]
                nc.vector.tensor_tensor(out=tsq, in0=yy, in1=yy,
                                        op=AluOpType.mult)
                nc.vector.tensor_tensor(out=tsq, in0=tsq,
                                        in1=hv[:], op=AluOpType.mult)
                nc.vector.tensor_scalar(out=tsq, in0=tsq,
                                        scalar1=-1.0, scalar2=1.5,
                                        op0=AluOpType.mult, op1=AluOpType.add)
                dst = gm[:, 0, :] if it == 1 else ynew
                nc.vector.tensor_tensor(out=dst, in0=yy, in1=tsq,
                                        op=AluOpType.mult)
                yy = ynew
            nc.vector.tensor_copy(out=gm[:, 1, :], in_=gmean)

            # expand group stats back to channels (tiny matmul with E)
            ps_e = psQ.tile([128, 8], F32, tag="ps", name="ps")
            nc.tensor.matmul(ps_e[:], lhsT=E_sb[:],
                             rhs=gm[:].rearrange("p s t -> p (s t)"),
                             start=True, stop=True)
            ab = sm.tile([128, 8], F32, tag="ab", name="ab")
            nc.vector.tensor_copy(out=ab[:], in_=ps_e[:])
            bvec = sm.tile([128, 4], F32, tag="bvec", name="bvec")
            nc.vector.tensor_tensor(out=bvec[:], in0=ab[:, 4:8], in1=ab[:, 0:4],
                                    op=AluOpType.mult)
            nc.vector.tensor_scalar_mul(bvec[:], bvec[:], -1.0)

            # xn (bf16) split across ACT and DVE so the critical path halves
            xn_sb = []
            for t in range(CT):
                xnt = big.tile([128, N], BF16, tag=f"xn{t}", name=f"xn{t}")
                if t % 2 == 0:
                    nc.scalar.activation(out=xnt[:], in_=xb_sb[t][:],
                                         func=AF.Identity,
                                         scale=ab[:, t:t + 1], bias=bvec[:, t:t + 1])
                else:
                    nc.vector.tensor_scalar(
                        out=xnt[:], in0=xb_sb[t][:],
                        scalar1=ab[:, t:t + 1], scalar2=bvec[:, t:t + 1],
                        op0=AluOpType.mult, op1=AluOpType.add)
                xn_sb.append(xnt)

            # ---- q/k projections (bf16 out) -----------------------------
            # consecutive matmuls share the stationary operand (ic inner)
            q_sb = [big.tile([128, N], BF16, tag=f"q{t}", name=f"q{t}")
                    for t in range(CT)]
            k_sb = [big.tile([128, N], BF16, tag=f"k{t}", name=f"k{t}")
                    for t in range(CT)]
            for ot in range(8):          # 0..3 -> q tiles, 4..7 -> k tiles
                dst = q_sb[ot] if ot < 4 else k_sb[ot - 4]
                ps2 = [psQ.tile([128, 512], F32, tag="ps", name="ps")
                       for _ in range(IC)]
                for kt in range(CT):
                    for ic in range(IC):
                        nc.tensor.matmul(
                            ps2[ic][:],
                            lhsT=wqk_sb[kt][:, 128 * ot:128 * (ot + 1)],
                            rhs=xn_sb[kt][:, 512 * ic:512 * (ic + 1)],
                            start=(kt == 0), stop=(kt == CT - 1))
                for ic in range(IC):
                    # ACT is idle pre-attention; fast PSUM drain keeps the PE
                    # from stalling on the 2-buffer rotation
                    nc.scalar.activation(
                        out=dst[:, 512 * ic:512 * (ic + 1)], in_=ps2[ic][:],
                        func=AF.Identity, bias=qkb_sb[:, ot:ot + 1])

            # ---- vT (pixels on partitions) + ones column ----------------
            vT_sb = []
            for jt in range(JT):
                sch = False
                vt = big.tile([128, NH, HD + 1], F32,
                              tag=f"vT{jt}", name=f"vT{jt}")
                psv = psQ.tile([128, 512], F32, tag="ps", name="ps")
                for kt in range(CT):
                    nc.tensor.matmul(
                        psv[:],
                        lhsT=xn_sb[kt][:, 128 * jt:128 * (jt + 1)],
                        rhs=wv_sb[kt][:],
                        start=(kt == 0), stop=(kt == CT - 1))
                dv = (lambda ap: ap) if sch else mm
                nc.vector.tensor_copy(
                    out=dv(vt[:, :, 0:HD]),
                    in_=psv[:].rearrange("p (h c) -> p h c", h=NH))
                nc.vector.tensor_copy(
                    out=dv(vt[:, :, HD:HD + 1]),
                    in_=ones_sb[:].to_broadcast((128, NH, 1)))
                vT_sb.append(vt)

            # ---- attention ---------------------------------------------
            ao_sb = [big.tile([128, N], BF16, tag=f"ao{t}", name=f"ao{t}")
                     for t in range(CT)]

            def finish_ht(ht, zd):
                # broadcast 1/Z rows of heads 2ht,2ht+1 across partitions via
                # the DRAM bounce, then normalize ao[ht] in place
                zb = zbp.tile([128, N], F32, tag="zb", name=f"zb{ht}")
                with tc.high_priority():
                    for hh in range(2):          # head 2*ht + hh
                        for ic in range(IC):
                            qb = nc.gpsimd if (hh + ic) % 2 == 0 else nc.sync
                            qb.dma_start(
                                out=zb[64 * hh:64 * (hh + 1),
                                       512 * ic:512 * (ic + 1)],
                                in_=zd[2 * hh + ic, :].partition_broadcast(64))
                    nc.vector.tensor_tensor(out=ao_sb[ht][:], in0=ao_sb[ht][:],
                                            in1=zb[:], op=AluOpType.mult)

            P_store = {}
            zd_store = {}

            def emit_qk(h):
                ht, hr = h // 2, (h % 2) * HD
                tiles = []
                for jt in range(JT):
                    psp = psA.tile([128, N], F32, tag="pp", name="pp")
                    for ic in range(IC):
                        nc.tensor.matmul(
                            psp[:, 512 * ic:512 * (ic + 1)],
                            lhsT=k_sb[ht][hr:hr + HD, 128 * jt:128 * (jt + 1)],
                            rhs=q_sb[ht][hr:hr + HD, 512 * ic:512 * (ic + 1)],
                            start=True, stop=True)
                    pt = ppool.tile([128, N], F32, tag="P", name="P")
                    nc.scalar.activation(out=mm(pt[:]), in_=psp[:],
                                         func=AF.Exp)
                    tiles.append(pt)
                P_store[h] = tiles

            def emit_pv(h):
                ht, hr = h // 2, (h % 2) * HD
                P_tiles = P_store.pop(h)
                pav = [psQ.tile([128, 512], F32, tag="ps", name="pav")
                       for _ in range(IC)]
                for jt in range(JT):
                    for ic in range(IC):
                        nc.tensor.matmul(
                            pav[ic][0:HD + 1, :],
                            lhsT=mm(vT_sb[jt][:, h, :]),
                            rhs=mm(P_tiles[jt][:, 512 * ic:512 * (ic + 1)]),
                            start=(jt == 0), stop=(jt == JT - 1))
                # drain PSUM immediately (high priority): o into ao
                # (unnormalized), 1/Z off the PSUM Z row via an SBUF stage;
                # pav recycles right away.
                if h % 2 == 0:
                    zd_store[ht] = zdp.tile([4, 512], F32, tag="zd",
                                            name=f"zd{ht}")
                zd_cur = zd_store[ht]
                with tc.high_priority():
                    for ic in range(IC):
                        nc.vector.tensor_copy(
                            out=ao_sb[ht][hr:hr + HD, 512 * ic:512 * (ic + 1)],
                            in_=pav[ic][0:HD, :])
                        zrow = zs.tile([1, 512], F32, tag="zrow", name="zrow")
                        nc.vector.tensor_copy(out=zrow[:], in_=pav[ic][HD:HD + 1, :])
                        rz = zs.tile([1, 512], F32, tag="rz", name="rz")
                        nc.vector.reciprocal_approx_fast(out=rz[:], in_=zrow[:])
                        qz = nc.sync if (h + ic) % 2 == 0 else nc.gpsimd
                        qz.dma_start(out=zd_cur[2 * (h % 2) + ic, :], in_=rz[:])
                if h % 2 == 1:
                    finish_ht(ht, zd_cur)

            # one-head-lag software pipeline: QK+exp of head h+1 is emitted
            # (and issues on the PE) before PV of head h, so the PE never
            # drains while ACT works through a head's 8 exps.
            emit_qk(0)
            for h in range(NH):
                if h + 1 < NH:
                    emit_qk(h + 1)
                emit_pv(h)

            # ---- projection + bias + residual --------------------------
            qo = [nc.sync, nc.gpsimd, nc.scalar]
            for ot in range(CT):
                y = big.tile([128, N], F32, tag=f"k{ot}", name=f"y{ot}",
                             padded_shape=[128, N])  # reuse dead k slot
                psj = [psA.tile([128, 512], F32, tag="pp", name="psj",
                                padded_shape=[128, N])
                       for _ in range(IC)]
                for ct in range(CT):
                    for ic in range(IC):
                        nc.tensor.matmul(
                            psj[ic][:],
                            lhsT=pw_sb[ct][:, 128 * ot:128 * (ot + 1)],
                            rhs=ao_sb[ct][:, 512 * ic:512 * (ic + 1)],
                            start=(ct == 0), stop=(ct == CT - 1))
                for ic in range(IC):
                    nc.vector.scalar_tensor_tensor(
                        out=y[:, 512 * ic:512 * (ic + 1)],
                        in0=psj[ic][:], scalar=pb_sb[:, ot:ot + 1],
                        in1=x_sb[ot][:, 512 * ic:512 * (ic + 1)],
                        op0=AluOpType.add, op1=AluOpType.add)
                qo[ot % 3].dma_start(out=io["out"][128 * ot:128 * (ot + 1), :],
                                     in_=y[:])


_NC_CACHE = {}


def _get_nc(p_bufs=17):
    key = p_bufs
    if key not in _NC_CACHE:
        nc = bacc.Bacc("TRN2", target_bir_lowering=False)
        io = _declare_io(nc)
        _build(nc, io, p_bufs=p_bufs)
        nc.compile()
        _NC_CACHE[key] = nc
    return _NC_CACHE[key]


def run(inputs, trace=False, **spmd_kwargs):
    """Build+run; returns (full_output, BassKernelResults)."""
    weights, xs, xbs = _host_prep(**inputs)
    nc = _get_nc()
    in_maps = [dict(weights, x=xs[b], xb=xbs[b]) for b in range(NCORES)]
    res = run_bass_kernel_spmd(nc, in_maps, list(range(NCORES)),
                               trace=trace, **spmd_kwargs)
    out = np.stack([res.results[b]["out"].reshape(C, 32, 32)
                    for b in range(NCORES)]).astype(np.float32)
    return out, res


def kernel(**inputs):
    out, _ = run(inputs, trace=False)
    return out


if __name__ == "__main__":
    rng = np.random.default_rng(0)
    demo = {
        "x": rng.standard_normal((8, 512, 32, 32), dtype=np.float32),
        "gn_gamma": np.ones(512, np.float32),
        "gn_beta": np.zeros(512, np.float32),
        "qkv_w": rng.standard_normal((1536, 512), dtype=np.float32) / 22.6,
        "qkv_b": rng.standard_normal(1536, dtype=np.float32) * 0.02,
        "proj_w": rng.standard_normal((512, 512), dtype=np.float32) / 22.6,
        "proj_b": rng.standard_normal(512, dtype=np.float32) * 0.02,
    }
    print(kernel(**demo).shape)
